# revision 1
# baseline (speedup 1.0000x reference)
"""CrossAttention on 8 TRN2 NeuronCores (tensor-parallel over heads).

Reference computation (B=4, N=2048, DIM=1024, 16 heads, head_dim=64):
    qkv = x @ Wqkv.T + bqkv ; q, k = split(qkv)  (v unused)
    attn = softmax(q @ k.T * scale) ; out = attn @ split_heads(context)
    return merge_heads(out) @ Wout.T + bout

Sharding: core c owns heads {2c, 2c+1}. Each core computes q/k
projections for its heads (full sequence), head-parallel attention with
context slices as values, then an AllToAll re-shards from head-parallel
to row-parallel so the output projection runs locally. Row ownership is
interleaved (core c owns rows [c*256:(c+1)*256] of every batch); the
re-shard is split into two collectives (batches 0-1 and 2-3) so the
first hides under the second half of attention and the second hides
under the output projection of the first batches.

All matmuls run in bf16 (fp32 PSUM accumulation); softmax runs exp on
ScalarE without max-subtraction (scores ~ N(0,1)), with the denominator
produced by an extra all-ones column appended to the value matrix.
The emission order software-pipelines the in-order engine streams:
qk-projection of batch b+1 is sliced into the attention groups of
batch b so ScalarE (the bottleneck) never starves.
"""
import numpy as np
import ml_dtypes

import concourse.bass as bass
import concourse.mybir as mybir
import concourse.tile as tile
from concourse import bacc
from concourse.bass_utils import run_bass_kernel_spmd

BF16 = ml_dtypes.bfloat16
F32 = mybir.dt.float32
BF = mybir.dt.bfloat16

NC = 8            # cores
B = 4             # batch
N = 2048          # sequence
DIM = 1024
NH = 16           # heads total
HD = 64           # head dim
HPC = NH // NC    # heads per core = 2
SCALE = HD ** -0.5
BN = B * N        # 8192 tokens
RPB = N // NC     # rows per (core, batch) after re-shard = 256
KC = DIM // 128   # contraction chunks for projections = 8
NKC = N // 128    # key chunks per batch = 16
CW = HD + 1       # value width incl. ones column = 65


def build(PIPELINE=True, NPHASE=2, MERGEH=False):
    QTAG = 2 if PIPELINE else B
    nc = bacc.Bacc("TRN2", target_bir_lowering=False, debug=False,
                   num_devices=NC)

    xT = nc.dram_tensor("xT", [DIM, BN], BF, kind="ExternalInput")
    wqkT = nc.dram_tensor("wqkT", [DIM, 2 * 128], BF, kind="ExternalInput")
    bqk = nc.dram_tensor("bqk", [2 * 128, 1], F32, kind="ExternalInput")
    ctxa = nc.dram_tensor("ctxa", [B, HPC, 128, NKC * CW], BF,
                          kind="ExternalInput")
    woutT = nc.dram_tensor("woutT", [DIM, DIM], BF, kind="ExternalInput")
    boutb = nc.dram_tensor("boutb", [128, DIM], F32, kind="ExternalInput")
    # out rows: batch-major, 256 rows per batch
    out = nc.dram_tensor("out", [B * RPB, DIM], F32, kind="ExternalOutput")

    # AllToAll bounce buffers, NPHASE collectives each covering B//NPHASE
    # batches; chunk j holds rows [j*256:(j+1)*256] of each covered batch
    bpp = B // NPHASE        # batches per phase
    a2a_in = [nc.dram_tensor(f"a2a_in{p}", [NC, 128, bpp * RPB], BF)
              for p in range(NPHASE)]
    a2a_out = [nc.dram_tensor(f"a2a_out{p}", [NC, 128, bpp * RPB], BF)
               for p in range(NPHASE)]

    rscr = [nc.dram_tensor(f"rscr{i}", [1, 512], F32) for i in range(8)]
    _scr_idx = [0]

    with tile.TileContext(nc) as tc:
        with tc.tile_pool(name="const", bufs=1) as const, \
             tc.tile_pool(name="qk", bufs=1) as qkpool, \
             tc.tile_pool(name="xt", bufs=8 if not PIPELINE else 10) as xtpool, \
             tc.tile_pool(name="pt", bufs=2) as ptpool, \
             tc.tile_pool(name="r1", bufs=4) as r1pool, \
             tc.tile_pool(name="rb", bufs=4) as rbpool, \
             tc.tile_pool(name="ho", bufs=4) as hopool, \
             tc.tile_pool(name="sl", bufs=16) as slpool, \
             tc.tile_pool(name="ob", bufs=4) as obpool, \
             tc.tile_pool(name="pc", bufs=3) as pcpool, \
             tc.tile_pool(name="pss", bufs=2, space="PSUM") as pss_pool, \
             tc.tile_pool(name="psm", bufs=4, space="PSUM") as psm_pool:

            # ---- small constants needed up front ----
            wqk_sb = []
            for kc in range(KC):
                t = const.tile([128, 256], BF, tag=f"wqk{kc}")
                nc.sync.dma_start(out=t[:], in_=wqkT[kc * 128:(kc + 1) * 128, :])
                wqk_sb.append(t)
            bq_sb = []
            for fb in range(2):
                t = const.tile([128, 1], F32, tag=f"bq{fb}")
                nc.sync.dma_start(out=t[:], in_=bqk[fb * 128:(fb + 1) * 128, :])
                bq_sb.append(t)

            wout_sb = []
            bout_sb = const.tile([128, DIM], F32, tag="bout")
            ctx_sb = {}
            qk_tiles = {}
            xt_tiles = {}

            def load_out_consts():
                for fc in range(KC):
                    t = const.tile([128, DIM], BF, tag=f"wout{fc}",
                                   name=f"wout{fc}")
                    nc.sync.dma_start(
                        out=t[:], in_=woutT[fc * 128:(fc + 1) * 128, :])
                    wout_sb.append(t)
                nc.sync.dma_start(out=bout_sb[:], in_=boutb[:])

            def load_ctx(b):
                for h in range(HPC):
                    t = const.tile([128, NKC * CW], BF, tag=f"ctx{b}{h}",
                                   name=f"ctx{b}_{h}")
                    nc.sync.dma_start(out=t[:], in_=ctxa[b, h, :, :])
                    ctx_sb[b, h] = t

            def prefetch_x(b):
                """Issue the xT DMAs and allocate q/k tiles for batch b."""
                qT = qkpool.tile([128, N], BF, tag=f"qT{b % QTAG}", name=f"qT{b}")
                kT = qkpool.tile([128, N], BF, tag=f"kT{b % QTAG}", name=f"kT{b}")
                qk_tiles[b] = (qT, kT)
                xts = []
                for kc in range(KC):
                    xt = xtpool.tile([128, N], BF, tag="xt",
                                     name=f"xtb{b}_{kc}")
                    nc.sync.dma_start(
                        out=xt[:], in_=xT[kc * 128:(kc + 1) * 128,
                                          b * N:(b + 1) * N])
                    xts.append(xt)
                xt_tiles[b] = xts

            def qkproj_slice(b, t):
                """Project token chunk t (512 tokens) of batch b."""
                qT, kT = qk_tiles[b]
                xts = xt_tiles[b]
                for fb, dst in ((1, kT), (0, qT)):
                    ps = psm_pool.tile([128, 512], F32, tag="psm",
                                       name=f"psq{b}_{t}_{fb}")
                    for kc in range(KC):
                        nc.tensor.matmul(
                            ps[:], wqk_sb[kc][:, fb * 128:(fb + 1) * 128],
                            xts[kc][:, t * 512:(t + 1) * 512],
                            start=(kc == 0), stop=(kc == KC - 1))
                    nc.vector.tensor_scalar_add(
                        dst[:, t * 512:(t + 1) * 512], ps[:], bq_sb[fb][:])

            def attention_group(b, h, qg):
                """Scores+softmax+values for one (head, 1024-query) group."""
                qT, kT = qk_tiles[b]
                hp = h * HD
                q0 = qg * 1024
                pt = ptpool.tile([128, NKC * 1024], BF, tag="pt",
                                 name=f"pt{b}_{h}_{qg}")
                for kc in range(NKC):
                    ps = pss_pool.tile([128, 1024], F32, tag="pss",
                                       name=f"pss{b}{h}{qg}{kc}")
                    for hf in range(2):
                        nc.tensor.matmul(
                            ps[:, hf * 512:(hf + 1) * 512],
                            kT[hp:hp + HD, kc * 128:(kc + 1) * 128],
                            qT[hp:hp + HD, q0 + hf * 512:q0 + (hf + 1) * 512],
                            start=True, stop=True)
                    nc.scalar.activation(
                        pt[:, kc * 1024:(kc + 1) * 1024], ps[:],
                        mybir.ActivationFunctionType.Exp, scale=SCALE)
                for qc in range(2):  # 512-query chunks
                    pav = psm_pool.tile([CW, 512], F32, tag="psm",
                                        name=f"pav{b}{h}{qg}{qc}")
                    for kc in range(NKC):
                        nc.tensor.matmul(
                            pav[:], ctx_sb[b, h][:, kc * CW:(kc + 1) * CW],
                            pt[:, kc * 1024 + qc * 512:
                               kc * 1024 + (qc + 1) * 512],
                            start=(kc == 0), stop=(kc == NKC - 1))
                    r1 = r1pool.tile([1, 512], F32, tag="r1",
                                     name=f"r1{b}{h}{qg}{qc}")
                    nc.vector.reciprocal(r1[:], pav[HD:CW, :])
                    # broadcast partition 0 -> 64 via a DRAM round-trip so
                    # gpsimd stays free to run collectives asynchronously
                    scr = rscr[_scr_idx[0] % 8]; _scr_idx[0] += 1
                    nc.sync.dma_start(out=scr[:], in_=r1[:])
                    rb = rbpool.tile([HD, 512], F32, tag="rb",
                                     name=f"rb{b}{h}{qg}{qc}")
                    nc.sync.dma_start(out=rb[:],
                                      in_=scr[:].broadcast_to([HD, 512]))
                    ho = hopool.tile([HD, 512], BF, tag="ho",
                                     name=f"ho{b}{h}{qg}{qc}")
                    nc.vector.tensor_tensor(
                        out=ho[:], in0=pav[0:HD, :], in1=rb[:],
                        op=mybir.AluOpType.mult)
                    # queries qq0..qq0+512 span two 256-row chunks
                    qq0 = q0 + qc * 512
                    for half in range(2):
                        j = (qq0 + half * 256) // RPB
                        o = (b % bpp) * RPB
                        nc.sync.dma_start(
                            out=a2a_in[b // bpp][j, h * HD:(h + 1) * HD,
                                                 o:o + RPB],
                            in_=ho[:, half * 256:(half + 1) * 256])

            def attention_pair(b, qg):
                """Both heads' scores+softmax+values for 512 queries.

                The two heads' score matmuls contract over disjoint
                row-groups of the PE array (partitions 0-63 / 64-127) and
                write disjoint PSUM banks, so they run concurrently.
                """
                qT, kT = qk_tiles[b]
                q0 = qg * 512
                pt = ptpool.tile([128, NKC * 1024], BF, tag="pt",
                                 name=f"ptp{b}_{qg}")
                for kc in range(NKC):
                    ps = pss_pool.tile([128, 1024], F32, tag="pss",
                                       name=f"pssp{b}{qg}{kc}")
                    for h in range(HPC):
                        nc.tensor.matmul(
                            ps[:, h * 512:(h + 1) * 512],
                            kT[h * HD:(h + 1) * HD, kc * 128:(kc + 1) * 128],
                            qT[h * HD:(h + 1) * HD, q0:q0 + 512],
                            start=True, stop=True,
                            tile_position=(h * HD, 0))
                    nc.scalar.activation(
                        pt[:, kc * 1024:(kc + 1) * 1024], ps[:],
                        mybir.ActivationFunctionType.Exp, scale=SCALE)
                for h in range(HPC):
                    pav = psm_pool.tile([CW, 512], F32, tag="psm",
                                        name=f"pavp{b}{qg}{h}")
                    for kc in range(NKC):
                        nc.tensor.matmul(
                            pav[:], ctx_sb[b, h][:, kc * CW:(kc + 1) * CW],
                            pt[:, kc * 1024 + h * 512:
                               kc * 1024 + (h + 1) * 512],
                            start=(kc == 0), stop=(kc == NKC - 1))
                    # evict PSUM immediately so the accumulator slot
                    # frees before the (long-latency) normalize chain
                    pc = pcpool.tile([CW, 512], F32, tag="pc",
                                     name=f"pcp{b}{qg}{h}")
                    nc.vector.tensor_copy(pc[:], pav[:])
                    r1 = r1pool.tile([1, 512], F32, tag="r1",
                                     name=f"r1p{b}{qg}{h}")
                    nc.vector.reciprocal(r1[:], pc[HD:CW, :])
                    scr = rscr[_scr_idx[0] % 8]; _scr_idx[0] += 1
                    nc.sync.dma_start(out=scr[:], in_=r1[:])
                    rb = rbpool.tile([HD, 512], F32, tag="rb",
                                     name=f"rbp{b}{qg}{h}")
                    nc.sync.dma_start(out=rb[:],
                                      in_=scr[:].broadcast_to([HD, 512]))
                    ho = hopool.tile([HD, 512], BF, tag="ho",
                                     name=f"hop{b}{qg}{h}")
                    nc.vector.tensor_tensor(
                        out=ho[:], in0=pc[0:HD, :], in1=rb[:],
                        op=mybir.AluOpType.mult)
                    for half in range(2):
                        j = (q0 + half * 256) // RPB
                        o = (b % bpp) * RPB
                        nc.sync.dma_start(
                            out=a2a_in[b // bpp][j, h * HD:(h + 1) * HD,
                                                 o:o + RPB],
                            in_=ho[:, half * 256:(half + 1) * 256])

            def reshard(p):
                nc.gpsimd.collective_compute(
                    "AllToAll", mybir.AluOpType.bypass,
                    replica_groups=[list(range(NC))],
                    ins=[a2a_in[p].ap().opt()], outs=[a2a_out[p].ap().opt()])

            def outproj(b):
                """Output projection for my 256 rows of batch b."""
                p, o = b // bpp, (b % bpp) * RPB
                for rc in range(RPB // 128):
                    sls = []
                    for fc in range(KC):
                        sl = slpool.tile([128, 128], BF, tag="sl",
                                         name=f"sl{b}_{rc}_{fc}")
                        nc.sync.dma_start(
                            out=sl[:],
                            in_=a2a_out[p][fc, :,
                                           o + rc * 128:o + (rc + 1) * 128])
                        sls.append(sl)
                    pso = [psm_pool.tile([128, 512], F32, tag="psm",
                                         name=f"pso{b}_{rc}_{i}")
                           for i in range(2)]
                    for fc in range(KC):
                        for n in range(2):
                            nc.tensor.matmul(
                                pso[n][:], sls[fc][:],
                                wout_sb[fc][:, n * 512:(n + 1) * 512],
                                start=(fc == 0), stop=(fc == KC - 1))
                    for n in range(2):
                        ob = obpool.tile([128, 512], F32, tag="ob",
                                         name=f"ob{b}_{rc}_{n}")
                        nc.vector.tensor_tensor(
                            out=ob[:], in0=pso[n][:],
                            in1=bout_sb[:, n * 512:(n + 1) * 512],
                            op=mybir.AluOpType.add)
                        nc.sync.dma_start(
                            out=out[b * RPB + rc * 128:
                                    b * RPB + (rc + 1) * 128,
                                    n * 512:(n + 1) * 512],
                            in_=ob[:])

            if PIPELINE:
                # software-pipelined emission
                prefetch_x(0)
                load_ctx(0)
                for t in range(4):
                    qkproj_slice(0, t)
                for b in range(B):
                    if b + 1 < B:
                        prefetch_x(b + 1)
                        load_ctx(b + 1)
                    for g, (h, qg) in enumerate(
                            ((0, 0), (0, 1), (1, 0), (1, 1))):
                        attention_group(b, h, qg)
                        if b + 1 < B:
                            qkproj_slice(b + 1, g)
                        elif g == 1:
                            load_out_consts()
                    if (b + 1) % bpp == 0:
                        reshard(b // bpp)
                for b in range(B):
                    outproj(b)
            else:
                # monolithic phases (v1-style), with the qk projection of
                # later batches staggered after earlier batches' attention
                # so the cold-clock ramp only fronts two batches of work
                for b in range(2):
                    prefetch_x(b)
                    load_ctx(b)
                    for t in range(4):
                        qkproj_slice(b, t)
                load_out_consts()
                for b in range(B):
                    if MERGEH:
                        if b + 2 < B:
                            prefetch_x(b + 2)
                            load_ctx(b + 2)
                            for t in range(4):
                                qkproj_slice(b + 2, t)
                        for qg in range(4):
                            attention_pair(b, qg)
                            # fill PE slack in the last batch with the
                            # output projection of the already-resharded
                            # first phase
                        if b == B - 1 and NPHASE > 1:
                            for bb in range(bpp):
                                outproj(bb)
                    else:
                        if b + 2 < B:
                            prefetch_x(b + 2)
                            load_ctx(b + 2)
                            for t in range(4):
                                qkproj_slice(b + 2, t)
                        for h, qg in ((0, 0), (0, 1), (1, 0), (1, 1)):
                            attention_group(b, h, qg)
                    if (b + 1) % bpp == 0:
                        reshard(b // bpp)
                for b in range(bpp if (MERGEH and NPHASE > 1) else 0, B):
                    outproj(b)
    nc.compile()
    return nc


def prep_inputs(x, context, Wqkv, bqkv, Wout, bout):
    """Host-side sharding: returns in_maps for the 8 cores."""
    x = np.asarray(x, np.float32)
    context = np.asarray(context, np.float32)
    Wqkv = np.asarray(Wqkv, np.float32)
    bqkv = np.asarray(bqkv, np.float32)
    Wout = np.asarray(Wout, np.float32)
    bout = np.asarray(bout, np.float32)

    xT = np.ascontiguousarray(x.reshape(BN, DIM).T).astype(BF16)
    woutT = np.ascontiguousarray(Wout.T).astype(BF16)
    boutb = np.broadcast_to(bout, (128, DIM)).astype(np.float32).copy()

    in_maps = []
    for c in range(NC):
        h0 = c * HPC
        # feature order: [q_h0 | q_h1] then [k_h0 | k_h1]
        wq = Wqkv[h0 * HD:(h0 + HPC) * HD]
        wk = Wqkv[DIM + h0 * HD:DIM + (h0 + HPC) * HD]
        wqkT = np.ascontiguousarray(
            np.concatenate([wq, wk], axis=0).T).astype(BF16)
        bq = np.concatenate([bqkv[h0 * HD:(h0 + HPC) * HD],
                             bqkv[DIM + h0 * HD:DIM + (h0 + HPC) * HD]])
        bq = bq.reshape(2 * 128, 1).astype(np.float32)
        ctxa = np.ones((B, HPC, 128, NKC, CW), np.float32)
        for h in range(HPC):
            g = h0 + h
            arr = context[:, :, g * HD:(g + 1) * HD].reshape(B, NKC, 128, HD)
            ctxa[:, h, :, :, :HD] = arr.transpose(0, 2, 1, 3)
        in_maps.append({
            "xT": xT,
            "wqkT": wqkT,
            "bqk": bq,
            "ctxa": ctxa.reshape(B, HPC, 128, NKC * CW).astype(BF16),
            "woutT": woutT,
            "boutb": boutb,
        })
    return in_maps


_NC_CACHE = None


import os


def _get_nc():
    global _NC_CACHE
    if _NC_CACHE is None:
        _NC_CACHE = build(
            PIPELINE=os.environ.get("K_PIPELINE", "0") == "1",
            NPHASE=int(os.environ.get("K_NPHASE", "2")),
            MERGEH=os.environ.get("K_MERGEH", "1") == "1")
    return _NC_CACHE


def run(in_maps, trace=False):
    nc = _get_nc()
    res = run_bass_kernel_spmd(nc, in_maps, core_ids=list(range(NC)),
                               trace=trace)
    # core c's out = [B*256, DIM]: rows [c*256:(c+1)*256] of each batch
    full = np.empty((B, N, DIM), np.float32)
    for c in range(NC):
        o = np.asarray(res.results[c]["out"]).reshape(B, RPB, DIM)
        full[:, c * RPB:(c + 1) * RPB, :] = o
    return full, res


def kernel(x, context, Wqkv, bqkv, Wout, bout):
    in_maps = prep_inputs(x, context, Wqkv, bqkv, Wout, bout)
    out, _ = run(in_maps, trace=False)
    return out



# revision 2
# speedup vs baseline: 1.0083x; 1.0083x over previous
"""CrossAttention on 8 TRN2 NeuronCores (tensor-parallel over heads).

Reference computation (B=4, N=2048, DIM=1024, 16 heads, head_dim=64):
    qkv = x @ Wqkv.T + bqkv ; q, k = split(qkv)  (v unused)
    attn = softmax(q @ k.T * scale) ; out = attn @ split_heads(context)
    return merge_heads(out) @ Wout.T + bout

Sharding: core c owns heads {2c, 2c+1}; an AllToAll per batch reshards
the head-parallel attention output to row-parallel (core c owns rows
[c*256:(c+1)*256] of every batch) for the local output projection.

Schedule: one globally software-pipelined slot loop over (group g =
512 queries, kc = 128-key chunk).  Each slot emits on PE: the value
matmul pair for group g-1, 0-3 "filler" matmuls (qk projection of the
next batch / output projection of a batch resharded TWO batches ago,
so collective latency is never exposed mid-stream), then the
tile-packed score matmul pair for (g, kc) — last, so its PSUM-ring
back-pressure wait never head-of-line blocks the other matmuls.
ScalarE runs one [128,1024] exp per slot and paces the pipeline from
~7us to ~380us at ~100% occupancy.  Softmax normalization: ones-column
denominators, PSUM eviction to bf16 SBUF, a rank-1 ones-matmul
partition broadcast, reciprocal_approx_fast, one multiply.  Scores,
qk-projection and output-projection accumulators share one 3-deep
PSUM ring (6 banks) sized so score matmuls throttle to the exp pace
with pre-satisfied semaphores; value accumulators (ones column
included) hold the remaining 2 banks.  Constants are single-DMA
host-relaid tensors; x tiles load one DMA per 128-row chunk (batch 0
split so the head projection starts after ~1MB lands); sl gathers ride
the Sync queue mid-stream and both queues at the tail.
"""
import numpy as np
import ml_dtypes

import concourse.bass as bass
import concourse.mybir as mybir
import concourse.tile as tile
from concourse import bacc
from concourse.bass_utils import run_bass_kernel_spmd

BF16 = ml_dtypes.bfloat16
F32 = mybir.dt.float32
BF = mybir.dt.bfloat16

NC = 8            # cores
B = 4             # batch
N = 2048          # sequence
DIM = 1024
NH = 16           # heads total
HD = 64           # head dim
HPC = NH // NC    # heads per core = 2
SCALE = HD ** -0.5
BN = B * N        # 8192 tokens
RPB = N // NC     # rows per (core, batch) after re-shard = 256
KC = DIM // 128   # contraction chunks for projections = 8
NKC = N // 128    # key chunks per batch = 16
CW = HD + 1       # value width incl. ones column = 65
NG = 4 * B        # 512-query groups total = 16


def build():
    nc = bacc.Bacc("TRN2", target_bir_lowering=False, debug=False,
                   num_devices=NC)

    xT = nc.dram_tensor("xT", [DIM, BN], BF, kind="ExternalInput")
    # constants pre-laid-out host-side so each is ONE contiguous DMA
    wqkT = nc.dram_tensor("wqkT", [128, KC * 256], BF, kind="ExternalInput")
    bqk = nc.dram_tensor("bqk", [128, 2], F32, kind="ExternalInput")
    ctxa = nc.dram_tensor("ctxa", [B, 128, HPC * NKC * CW], BF,
                          kind="ExternalInput")
    woutT = nc.dram_tensor("woutT", [128, KC * DIM], BF,
                           kind="ExternalInput")
    boutb = nc.dram_tensor("boutb", [128, DIM], F32, kind="ExternalInput")
    out = nc.dram_tensor("out", [B * RPB, DIM], F32, kind="ExternalOutput")

    a2a_in = [nc.dram_tensor(f"a2a_in{p}", [NC, 128, RPB], BF)
              for p in range(B)]
    a2a_out = [nc.dram_tensor(f"a2a_out{p}", [NC, 128, RPB], BF)
               for p in range(B)]

    with tile.TileContext(nc) as tc:
        with tc.tile_pool(name="const", bufs=1) as const, \
             tc.tile_pool(name="qk", bufs=1) as qkpool, \
             tc.tile_pool(name="xt", bufs=8) as xtpool, \
             tc.tile_pool(name="xth", bufs=8) as xthpool, \
             tc.tile_pool(name="xtr", bufs=8) as xtrpool, \
             tc.tile_pool(name="pt", bufs=18) as ptpool, \
             tc.tile_pool(name="pc", bufs=3) as pcpool, \
             tc.tile_pool(name="ho", bufs=3) as hopool, \
             tc.tile_pool(name="sl", bufs=16) as slpool, \
             tc.tile_pool(name="ob", bufs=4) as obpool, \
             tc.tile_pool(name="rr", bufs=2) as rrpool, \
             tc.tile_pool(name="ring", bufs=3, space="PSUM") as ring_pool, \
             tc.tile_pool(name="psv", bufs=2, space="PSUM") as psv_pool:

            # ---- constants (one DMA each) ----
            wqk_sb = const.tile([128, KC * 256], BF, tag="wqk")
            nc.sync.dma_start(out=wqk_sb[:], in_=wqkT[:])
            bq_sb = const.tile([128, 2], F32, tag="bq")
            nc.sync.dma_start(out=bq_sb[:], in_=bqk[:])
            ones_sb = const.tile([128, HD], BF, tag="ones")
            nc.vector.memset(ones_sb[:], 1.0)
            wout_sb = const.tile([128, KC * DIM], BF, tag="wout")
            bout_sb = const.tile([128, DIM], F32, tag="bout")

            def load_out_consts():
                nc.sync.dma_start(out=wout_sb[:], in_=woutT[:])
                nc.sync.dma_start(out=bout_sb[:], in_=boutb[:])

            ctx_sb = {}

            def load_ctx(b):
                t = const.tile([128, HPC * NKC * CW], BF, tag=f"ctx{b}")
                nc.sync.dma_start(out=t[:], in_=ctxa[b, :, :])
                ctx_sb[b] = t

            # x tiles: [128, 2048] per (batch, kc); one DMA each so the
            # Sync sequencer isn't swamped with trigger setup time.
            # batch 0 is split t0 / t1-3 so the head projection only waits
            # for the first quarter of the batch to land.
            xt_tiles = {}

            def load_xt(b, split=False):
                if split:
                    for kc in range(KC):
                        xt = xthpool.tile([128, 512], BF, tag="xth",
                                         name=f"xth{b}_{kc}")
                        nc.sync.dma_start(
                            out=xt[:],
                            in_=xT[kc * 128:(kc + 1) * 128,
                                   b * N:b * N + 512])
                        xt_tiles[b, kc, 0] = xt
                    for kc in range(KC):
                        xt = xtrpool.tile([128, 1536], BF, tag="xtr",
                                         name=f"xtr{b}_{kc}")
                        nc.sync.dma_start(
                            out=xt[:],
                            in_=xT[kc * 128:(kc + 1) * 128,
                                   b * N + 512:(b + 1) * N])
                        xt_tiles[b, kc, 1] = xt
                else:
                    for kc in range(KC):
                        xt = xtpool.tile([128, N], BF, tag="xt",
                                         name=f"xt{b}_{kc}")
                        nc.sync.dma_start(
                            out=xt[:],
                            in_=xT[kc * 128:(kc + 1) * 128,
                                   b * N:(b + 1) * N])
                        xt_tiles[b, kc] = xt

            def xslice(b, kc, t):
                """[128, 512] token-slice t of batch b's chunk kc."""
                if (b, kc) in xt_tiles:
                    return xt_tiles[b, kc][:, t * 512:(t + 1) * 512]
                if t == 0:
                    return xt_tiles[b, kc, 0][:]
                return xt_tiles[b, kc, 1][:, (t - 1) * 512:t * 512]

            def xslice2(b, kc, t2):
                """[128, 1024] token-half t2 (batches with whole tiles)."""
                return xt_tiles[b, kc][:, t2 * 1024:(t2 + 1) * 1024]

            qk_tiles = {}

            def alloc_qk(b):
                qT = qkpool.tile([128, N], BF, tag=f"qT{b % 2}", name=f"qT{b}")
                kT = qkpool.tile([128, N], BF, tag=f"kT{b % 2}", name=f"kT{b}")
                qk_tiles[b] = (qT, kT)

            # ---- qk-projection fillers ----
            # batch-0 form: 4-matmul N=512 unit; two consecutive units
            # make one (t, fb) accumulation group + bias-add evict
            psq_cur = [None]

            def qk_filler5(b, t, fb, half):
                if half == 0:
                    psq_cur[0] = ring_pool.tile([128, 512], F32, tag="ring",
                                                name=f"psq{b}_{t}_{fb}")
                ps = psq_cur[0]
                for kc in range(half * 4, half * 4 + 4):
                    nc.tensor.matmul(
                        ps[:], wqk_sb[:, kc * 256 + fb * 128:
                                      kc * 256 + (fb + 1) * 128],
                        xslice(b, kc, t),
                        start=(kc == 0), stop=(kc == KC - 1))
                if half == 1:
                    qT, kT = qk_tiles[b]
                    dst = kT if fb == 1 else qT
                    nc.vector.tensor_scalar_add(
                        dst[:, t * 512:(t + 1) * 512], ps[:],
                        bq_sb[:, fb:fb + 1])

            # steady-state form: same 4-matmul N=512 units for the
            # other batches (reading whole [128,2048] x tiles)
            def qk_filler6(b, fb, t2, part, half):
                t = t2 * 2 + part
                if half == 0:
                    psq_cur[0] = ring_pool.tile([128, 512], F32, tag="ring",
                                                name=f"psq{b}_{t}_{fb}")
                ps = psq_cur[0]
                for kc in range(half * 4, half * 4 + 4):
                    nc.tensor.matmul(
                        ps[:], wqk_sb[:, kc * 256 + fb * 128:
                                      kc * 256 + (fb + 1) * 128],
                        xslice(b, kc, t),
                        start=(kc == 0), stop=(kc == KC - 1))
                if half == 1:
                    qT, kT = qk_tiles[b]
                    dst = kT if fb == 1 else qT
                    nc.vector.tensor_scalar_add(
                        dst[:, t * 512:(t + 1) * 512], ps[:],
                        bq_sb[:, fb:fb + 1])

            # ---- output-projection fillers ----
            # 4-matmul N=512 units; per (b, rc): 8 sl loads with the first
            # unit, two units per n-half, bias + store after each n-half.
            # sl loads ride the Sync queue: outproj runs two batches after
            # its reshard, so the collective-done wait is long satisfied.
            sl_cur = {}
            pso_cur = [None]

            def op_filler(b, rc, n, half, q=None):
                if n == 0 and half == 0:
                    sls = []
                    for f in range(KC):
                        sl = slpool.tile([128, 128], BF, tag="sl",
                                         name=f"sl{b}_{rc}_{f}")
                        (q or nc.sync).dma_start(
                            out=sl[:],
                            in_=a2a_out[b][f, :, rc * 128:(rc + 1) * 128])
                        sls.append(sl)
                    sl_cur[b, rc] = sls
                if half == 0:
                    pso_cur[0] = ring_pool.tile([128, 512], F32, tag="ring",
                                                name=f"pso{b}_{rc}_{n}")
                ps = pso_cur[0]
                for fc in range(half * 4, half * 4 + 4):
                    nc.tensor.matmul(
                        ps[:], sl_cur[b, rc][fc][:],
                        wout_sb[:, fc * 1024 + n * 512:
                                fc * 1024 + (n + 1) * 512],
                        start=(fc == 0), stop=(fc == KC - 1))
                if half == 1:
                    ob = obpool.tile([128, 512], F32, tag="ob",
                                     name=f"ob{b}_{rc}_{n}")
                    nc.vector.tensor_tensor(
                        out=ob[:], in0=ps[:],
                        in1=bout_sb[:, n * 512:(n + 1) * 512],
                        op=mybir.AluOpType.add)
                    nc.sync.dma_start(
                        out=out[b * RPB + rc * 128:b * RPB + (rc + 1) * 128,
                                n * 512:(n + 1) * 512],
                        in_=ob[:])

            # ---- attention pieces ----
            pt_tiles = {}
            pav_tiles = {}

            def scores_exp(g, kc):
                b, qg = g // 4, g % 4
                qT, kT = qk_tiles[b]
                q0 = qg * 512
                ps = ring_pool.tile([128, 1024], F32, tag="ring",
                                    name=f"pss{g}_{kc}")
                for h in range(HPC):
                    nc.tensor.matmul(
                        ps[:, h * 512:(h + 1) * 512],
                        kT[h * HD:(h + 1) * HD, kc * 128:(kc + 1) * 128],
                        qT[h * HD:(h + 1) * HD, q0:q0 + 512],
                        start=True, stop=True,
                        tile_position=(h * HD, 0))
                pt = ptpool.tile([128, 1024], BF, tag="pt",
                                 name=f"pt{g}_{kc}")
                nc.scalar.activation(
                    pt[:], ps[:], mybir.ActivationFunctionType.Exp,
                    scale=SCALE)
                pt_tiles[g, kc] = pt

            def vals(g, kc):
                b = g // 4
                if kc == 0:
                    for h in range(HPC):
                        pav_tiles[g, h] = psv_pool.tile(
                            [CW, 512], F32, tag="psv", name=f"pav{g}_{h}")
                for h in range(HPC):
                    nc.tensor.matmul(
                        pav_tiles[g, h][:],
                        ctx_sb[b][:, (h * NKC + kc) * CW:
                                  (h * NKC + kc + 1) * CW],
                        pt_tiles[g, kc][:, h * 512:(h + 1) * 512],
                        start=(kc == 0), stop=(kc == NKC - 1))

            def normalize(g):
                b, qg = g // 4, g % 4
                # evict both heads' PSUM accumulators to one SBUF tile
                # (bf16 — the output is rounded to bf16 anyway) so the next
                # group's value matmuls never block on the normalize chain;
                # a rank-1 ones matmul broadcasts the denominator row to 64
                # partitions in PSUM (no DMA round-trip on the chain)
                pc = pcpool.tile([CW, 1024], BF, tag="pc", name=f"pc{g}")
                for h in range(HPC):
                    nc.vector.tensor_copy(
                        pc[:, h * 512:(h + 1) * 512], pav_tiles[g, h][:])
                rb = ring_pool.tile([HD, 1024], F32, tag="ring",
                                    name=f"rbc{g}")
                for j in range(2):
                    nc.tensor.matmul(
                        rb[:, j * 512:(j + 1) * 512],
                        ones_sb[HD:HD + 1, 0:HD],
                        pc[HD:CW, j * 512:(j + 1) * 512],
                        start=True, stop=True, tile_position=(64, 0))
                rr = rrpool.tile([HD, 1024], F32, tag="rr", name=f"rr{g}")
                nc.vector.reciprocal_approx_fast(rr[:], rb[:])
                ho = hopool.tile([HD, 1024], BF, tag="ho", name=f"ho{g}")
                nc.vector.tensor_tensor(
                    out=ho[:], in0=pc[0:HD, :], in1=rr[:],
                    op=mybir.AluOpType.mult)
                for h in range(HPC):
                    for half in range(2):
                        j = qg * 2 + half
                        nc.sync.dma_start(
                            out=a2a_in[b][j, h * HD:(h + 1) * HD, :],
                            in_=ho[:, h * 512 + half * 256:
                                   h * 512 + (half + 1) * 256])

            def reshard(p):
                nc.gpsimd.collective_compute(
                    "AllToAll", mybir.AluOpType.bypass,
                    replica_groups=[list(range(NC))],
                    ins=[a2a_in[p].ap().opt()], outs=[a2a_out[p].ap().opt()])

            # ---- filler queues ----
            # qk5 (batch 0): 4-mm N=512 units in (half0, half1) pairs on
            # adjacent slots; qk6/op6: 2-mm N=1024 units in quads of 4
            # consecutive slots
            def qk5_units(b, pairs):
                return [('qk5', b, t, fb, half) for (t, fb) in pairs
                        for half in range(2)]

            def qk6_units(b):
                return [('qk6', b, fb, t2, part, half)
                        for fb in (1, 0) for t2 in range(2)
                        for part in range(2) for half in range(2)]

            def op6_units(b):
                return [('op6', b, rc, n, half) for rc in range(2)
                        for n in range(2) for half in range(2)]

            REST0 = [(1, 1), (2, 1), (3, 1), (1, 0), (2, 0), (3, 0)]

            def spread_runs(units, starts, run):
                """consecutive runs of `run` units land on consecutive
                slots starting at each start, so ring tiles close fast."""
                sched = {}
                for i, u in enumerate(units):
                    s = starts[i // run] + i % run
                    sched.setdefault(s, []).append(u)
                return sched

            def merge(a, b):
                for s, us in b.items():
                    a.setdefault(s, []).extend(us)
                return a

            fills = {}
            fills[0] = merge(
                spread_runs(qk5_units(0, REST0), list(range(0, 12, 2)), 2),
                spread_runs(qk6_units(1), list(range(14, 62, 6)), 2))
            # output projections run TWO batches after their data batch so
            # the reshard collective always finishes long before its
            # consumers reach the PE stream
            fills[1] = spread_runs(qk6_units(2), list(range(2, 42, 5)), 2)
            fills[2] = merge(
                spread_runs(qk6_units(3), list(range(2, 42, 5)), 2),
                spread_runs(op6_units(0), [44, 48, 52, 56], 2))
            fills[3] = merge(
                spread_runs(op6_units(1), [14, 18, 22, 26], 2),
                spread_runs(op6_units(2), [36, 42, 48, 54], 2))
            def run_filler(u):
                if u[0] == 'qk5':
                    qk_filler5(*u[1:])
                elif u[0] == 'qk6':
                    qk_filler6(*u[1:])
                else:
                    op_filler(*u[1:])

            # ---- emission ----
            load_xt(0, split=True)
            load_ctx(0)
            alloc_qk(0)
            for u in qk5_units(0, [(0, 1), (0, 0)]):
                run_filler(u)

            for b in range(B):
                if b == 0:
                    load_out_consts()
                if b + 1 < B:
                    load_xt(b + 1)
                    load_ctx(b + 1)
                    alloc_qk(b + 1)
                fq = fills[b]
                for gq in range(4):
                    g = 4 * b + gq
                    for kc in range(NKC):
                        if g >= 1:
                            vals(g - 1, kc)
                        for u in fq.get(gq * NKC + kc, []):
                            run_filler(u)
                        scores_exp(g, kc)
                    if g >= 1:
                        normalize(g - 1)
                        if (g - 1) % 4 == 3:
                            reshard((g - 1) // 4)

            # drain: values + normalize of the last group, final reshard,
            # output projection of batch 3 (rc1's sl loads on the gpsimd
            # queue so the two rc's loads issue in parallel at the tail)
            for kc in range(NKC):
                vals(NG - 1, kc)
            normalize(NG - 1)
            reshard(3)
            for u in op6_units(3):
                if u[2] == 1 and u[3] == 0 and u[4] == 0:
                    op_filler(*u[1:], q=nc.gpsimd)
                else:
                    op_filler(*u[1:])

    nc.compile()
    return nc


def prep_inputs(x, context, Wqkv, bqkv, Wout, bout):
    """Host-side sharding: returns in_maps for the 8 cores."""
    x = np.asarray(x, np.float32)
    context = np.asarray(context, np.float32)
    Wqkv = np.asarray(Wqkv, np.float32)
    bqkv = np.asarray(bqkv, np.float32)
    Wout = np.asarray(Wout, np.float32)
    bout = np.asarray(bout, np.float32)

    xT = np.ascontiguousarray(x.reshape(BN, DIM).T).astype(BF16)
    # wout laid out as [128, KC*1024]: chunk fc on cols [fc*1024:(fc+1)*1024]
    wT = np.ascontiguousarray(Wout.T).astype(BF16)
    woutT = np.concatenate(
        [wT[fc * 128:(fc + 1) * 128, :] for fc in range(KC)], axis=1)
    woutT = np.ascontiguousarray(woutT)
    boutb = np.broadcast_to(bout, (128, DIM)).astype(np.float32).copy()

    in_maps = []
    for c in range(NC):
        h0 = c * HPC
        wq = Wqkv[h0 * HD:(h0 + HPC) * HD]
        wk = Wqkv[DIM + h0 * HD:DIM + (h0 + HPC) * HD]
        w = np.ascontiguousarray(
            np.concatenate([wq, wk], axis=0).T).astype(BF16)  # [1024, 256]
        # [128, KC*256]: chunk kc on cols [kc*256:(kc+1)*256]
        wqkT_c = np.ascontiguousarray(np.concatenate(
            [w[kc * 128:(kc + 1) * 128, :] for kc in range(KC)], axis=1))
        bq = np.stack([bqkv[h0 * HD:(h0 + HPC) * HD],
                       bqkv[DIM + h0 * HD:DIM + (h0 + HPC) * HD]],
                      axis=1).astype(np.float32)  # [128, 2]
        ctxa = np.ones((B, HPC, 128, NKC, CW), np.float32)
        for h in range(HPC):
            g = h0 + h
            arr = context[:, :, g * HD:(g + 1) * HD].reshape(B, NKC, 128, HD)
            ctxa[:, h, :, :, :HD] = arr.transpose(0, 2, 1, 3)
        # [B, 128, HPC*NKC*CW]: head h on cols [h*NKC*CW:(h+1)*NKC*CW]
        ctxa = ctxa.transpose(0, 2, 1, 3, 4).reshape(
            B, 128, HPC * NKC * CW).astype(BF16)
        in_maps.append({
            "xT": xT,
            "wqkT": wqkT_c,
            "bqk": np.ascontiguousarray(bq),
            "ctxa": np.ascontiguousarray(ctxa),
            "woutT": woutT,
            "boutb": boutb,
        })
    return in_maps


_NC_CACHE = None


def _get_nc():
    global _NC_CACHE
    if _NC_CACHE is None:
        _NC_CACHE = build()
    return _NC_CACHE


def run(in_maps, trace=False):
    nc = _get_nc()
    res = run_bass_kernel_spmd(nc, in_maps, core_ids=list(range(NC)),
                               trace=trace)
    full = np.empty((B, N, DIM), np.float32)
    for c in range(NC):
        o = np.asarray(res.results[c]["out"]).reshape(B, RPB, DIM)
        full[:, c * RPB:(c + 1) * RPB, :] = o
    return full, res


def kernel(x, context, Wqkv, bqkv, Wout, bout):
    in_maps = prep_inputs(x, context, Wqkv, bqkv, Wout, bout)
    out, _ = run(in_maps, trace=False)
    return out
